# revision 8
# baseline (speedup 1.0000x reference)
"""Trainium2 Bass kernel for nn_GenerativeODE (VAE encoder + RK4 neural-ODE +
zone decoder), data-parallel over 8 NeuronCores (batch 4096 -> 512/core).

Layout: feature-major on device ([feature, sample]); batch 512/core processed
as 2 interleaved substreams of 256 so ScalarE (tanh) and TensorE overlap.

Key restructuring (validated in numpy to 8e-7 rel err):
  - ode_Wo is folded into the next RK4 stage's input matmul:
      Wi_z.T @ (z + a*k_prev) = Wi_z.T @ z + a*(Wo@Wi_z).T @ hh_prev + const
    so k tensors are never materialized; everything accumulates in PSUM.
  - time/bias terms are per-(step, stage) per-partition bias vectors applied
    for free by the ACTIVATE instruction (tanh(scale*x + bias)).
  - z_{t+1} = z_t + Wo6.T@hh1 + Wo3.T@hh2 + Wo3.T@hh3 + Wo6.T@hh4 (+dt*bo)
    accumulated on the PE via an identity matmul for the z term.
  - decoder runs batch-major (lhsT = z chunk) so logits DMA out is contiguous
    4KB rows; overlaps the next ODE step.
"""
import os
import sys
import numpy as np

for _p in ("/opt/trn_rl_repo", "/root/.axon_site/_ro/trn_rl_repo"):
    if os.path.isdir(_p) and _p not in sys.path:
        sys.path.append(_p)

import concourse.bass as bass
import concourse.mybir as mybir
import concourse.tile as tile

F32 = mybir.dt.float32
AF = mybir.ActivationFunctionType

N_CORES = 8
B, P, Z, ZE, PG, EH, L, OH, T = 4096, 32, 1000, 64, 8, 256, 64, 128, 48
BS = B // N_CORES          # 512 per core
NST = int(os.environ.get("KNST", "1"))  # substreams
SKIP_PE = os.environ.get("KSKIP", "pe") == "pe"  # resblock skip-add engine
VARIANT = NST + (8 if SKIP_PE else 0)
FD = BS // NST             # 256
NSTEP = T - 1              # 47

_wsplit_counter = [0]


def _split_excess_waits(nc, max_waits=1):
    """walrus CoreV2/V3 codegen allows only one sync-wait per instruction;
    split extra waits onto NoOps inserted before the offending instruction."""
    for fn in nc.m.functions:
        for bb in fn.blocks:
            insts = bb.instructions
            if not any(
                getattr(i, "sync_info", None) is not None
                and i.sync_info is not None
                and len(i.sync_info.on_wait) > max_waits
                for i in insts
            ):
                continue
            new = []
            for ins in insts:
                si = getattr(ins, "sync_info", None)
                if si is not None and len(si.on_wait) > max_waits:
                    waits = list(si.on_wait)
                    while len(waits) > max_waits:
                        chunk, waits = waits[:max_waits], waits[max_waits:]
                        _wsplit_counter[0] += 1
                        nop = mybir.InstNoOp(name=f"I-wsplit-{_wsplit_counter[0]}")
                        nop.engine = ins.engine
                        nop.sync_info = mybir.SyncInfo(on_wait=chunk, on_update=[])
                        new.append(nop)
                    ins.sync_info = mybir.SyncInfo(
                        on_wait=waits, on_update=list(si.on_update)
                    )
                new.append(ins)
            bb.instructions = new


def _build_program():
    nc = bass.Bass("TRN2", target_bir_lowering=False, debug=False)
    D = lambda name, shape: nc.dram_tensor(name, list(shape), F32, kind="ExternalInput")
    # per-core tensors
    encCatA = D("encCatA", (128, BS))
    encCatB = D("encCatB", (40, BS))
    catCA = D("catCA", (128, BS))
    catCB = D("catCB", (32, BS))
    epsT = D("epsT", (L, BS))
    btab = D("btab", (128, NSTEP * 3 + VARIANT))
    # replicated weights
    w1a = D("w1a", (128, EH))
    w1b = D("w1b", (40, EH))
    b1c = D("b1c", (128, 2))          # enc_b1 as two [128,1] columns
    w2 = D("w2", (EH, 128))
    b2mu = D("b2mu", (L, 1))
    b2lvh = D("b2lvh", (L, 1))
    wica = D("wica", (128, OH))
    wicb = D("wicb", (32, OH))
    i128 = D("i128", (128, 128))
    i64 = D("i64", (L, L))
    wiz = D("wiz", (L, OH))
    wfh = D("wfh", (OH, OH))
    wff = D("wff", (OH, OH))
    wz6 = D("wz6", (OH, OH))
    wz3 = D("wz3", (OH, OH))
    rb1w1 = D("rb1w1", (OH, OH))
    rb1w2 = D("rb1w2", (OH, OH))
    rb2w1 = D("rb2w1", (OH, OH))
    rb2w2 = D("rb2w2", (OH, OH))
    rbb = D("rbb", (OH, 4))
    wo6 = D("wo6", (OH, L))
    wo3 = D("wo3", (OH, L))
    bov = D("bov", (L, 1))
    decwa = D("decwa", (L, 512))
    decwb = D("decwb", (L, 512))
    decbb = D("decbb", (128, 1024))

    logits = nc.dram_tensor("logits", [BS, T, Z], F32, kind="ExternalOutput")
    muT = nc.dram_tensor("muT", [L, BS], F32, kind="ExternalOutput")
    lvT = nc.dram_tensor("lvT", [L, BS], F32, kind="ExternalOutput")

    with tile.TileContext(nc) as tc:
        with tc.tile_pool(name="const", bufs=1) as cp, \
             tc.tile_pool(name="zpool", bufs=3) as zp, \
             tc.tile_pool(name="work", bufs=4) as wp, \
             tc.tile_pool(name="stage", bufs=3) as sp, \
             tc.tile_pool(name="xps", bufs=3, space="PSUM") as xpp, \
             tc.tile_pool(name="znps", bufs=1, space="PSUM") as znpp, \
             tc.tile_pool(name="dps", bufs=2, space="PSUM") as dpp, \
             tc.tile_pool(name="decps", bufs=1, space="PSUM") as decp:

            # ---- load constants into SBUF ----
            def ld(dram, shape, name):
                t_ = cp.tile(list(shape), F32, name=name, tag=name)
                nc.sync.dma_start(t_[:], dram.ap())
                return t_

            s_encCatA = ld(encCatA, (128, BS), "s_encCatA")
            s_encCatB = ld(encCatB, (40, BS), "s_encCatB")
            s_catCA = ld(catCA, (128, BS), "s_catCA")
            s_catCB = ld(catCB, (32, BS), "s_catCB")
            s_epsT = ld(epsT, (L, BS), "s_epsT")
            s_btab = ld(btab, (128, NSTEP * 3 + VARIANT), "s_btab")
            s_w1a = ld(w1a, (128, EH), "s_w1a")
            s_w1b = ld(w1b, (40, EH), "s_w1b")
            s_b1c = ld(b1c, (128, 2), "s_b1c")
            # enc_W2 is [256,128]: SBUF tiles cap at 128 partitions, so load
            # it as two K-chunks.
            s_w2 = cp.tile([128, 128], F32, name="s_w2", tag="s_w2")
            nc.sync.dma_start(s_w2[:], w2.ap()[0:128, :])
            s_b2mu = ld(b2mu, (L, 1), "s_b2mu")
            s_b2lvh = ld(b2lvh, (L, 1), "s_b2lvh")
            s_wica = ld(wica, (128, OH), "s_wica")
            s_wicb = ld(wicb, (32, OH), "s_wicb")
            s_i128 = ld(i128, (128, 128), "s_i128")
            s_i64 = ld(i64, (L, L), "s_i64")
            s_wiz = ld(wiz, (L, OH), "s_wiz")
            s_wfh = ld(wfh, (OH, OH), "s_wfh")
            s_wff = ld(wff, (OH, OH), "s_wff")
            s_wz6 = ld(wz6, (OH, OH), "s_wz6")
            s_wz3 = ld(wz3, (OH, OH), "s_wz3")
            s_rb = [
                ld(rb1w1, (OH, OH), "s_rb1w1"),
                ld(rb1w2, (OH, OH), "s_rb1w2"),
                ld(rb2w1, (OH, OH), "s_rb2w1"),
                ld(rb2w2, (OH, OH), "s_rb2w2"),
            ]
            s_rbb = ld(rbb, (OH, 4), "s_rbb")
            s_wo6 = ld(wo6, (OH, L), "s_wo6")
            s_wo3 = ld(wo3, (OH, L), "s_wo3")
            s_bov = ld(bov, (L, 1), "s_bov")
            s_decwa = ld(decwa, (L, 512), "s_decwa")
            s_decwb = ld(decwb, (L, 512), "s_decwb")
            s_decbb = ld(decbb, (128, 1024), "s_decbb")

            # ---- encoder ----
            hs = []
            for m in range(2):
                hp = decp.tile([128, 1024], F32, name=f"hp{m}", tag="dec")
                nc.tensor.matmul(out=hp[:, 0:512], lhsT=s_w1a[:, m * 128:(m + 1) * 128],
                                 rhs=s_encCatA[:], start=True, stop=False)
                nc.tensor.matmul(out=hp[:, 0:512], lhsT=s_w1b[:, m * 128:(m + 1) * 128],
                                 rhs=s_encCatB[:], start=False, stop=True)
                h = wp.tile([128, BS], F32, name=f"h{m}", tag=f"ench{m}")
                nc.scalar.activation(h[:], hp[:, 0:512], AF.Relu, bias=s_b1c[:, m:m + 1])
                hs.append(h)
            lp = decp.tile([128, 1024], F32, name="lp", tag="dec")
            nc.tensor.matmul(out=lp[:, 0:512], lhsT=s_w2[:], rhs=hs[0][:],
                             start=True, stop=False)
            # second K-chunk of enc_W2 lives in rows 128:256 -> separate tile
            # (SBUF tiles are <=128 partitions, so w2 is passed as [256,128]
            # DRAM and loaded into two tiles)
            s_w2b = cp.tile([128, 128], F32, name="s_w2b", tag="s_w2b")
            nc.sync.dma_start(s_w2b[:], w2.ap()[128:256, :])
            nc.tensor.matmul(out=lp[:, 0:512], lhsT=s_w2b[:], rhs=hs[1][:],
                             start=False, stop=True)

            mu_sb = wp.tile([L, BS], F32, name="mu_sb", tag="mu_sb")
            nc.vector.tensor_scalar_add(mu_sb[:], lp[0:64, 0:512], s_b2mu[:, 0:1])
            nc.sync.dma_start(muT.ap(), mu_sb[:])
            lv_sb = wp.tile([L, BS], F32, name="lv_sb", tag="lv_sb")
            nc.vector.tensor_scalar_add(lv_sb[:], lp[64:128, 0:512], s_b2lvh[:, 0:1])
            # lvT output: b2lvh holds 0.5*b2lv, so add it twice for the raw
            # log_var output (lv = lp + b2lv = lp + 2*(0.5 b2lv)).
            lv_out = wp.tile([L, BS], F32, name="lv_out", tag="lv_out")
            nc.vector.tensor_scalar_add(lv_out[:], lv_sb[:], s_b2lvh[:, 0:1])
            nc.sync.dma_start(lvT.ap(), lv_out[:])
            std_sb = wp.tile([L, BS], F32, name="std_sb", tag="std_sb")
            nc.scalar.activation(std_sb[:], lp[64:128, 0:512], AF.Exp,
                                 bias=s_b2lvh[:, 0:1], scale=0.5)
            et = wp.tile([L, BS], F32, name="et", tag="et")
            nc.vector.tensor_mul(et[:], s_epsT[:], std_sb[:])
            z_cur = zp.tile([L, BS], F32, name="z0", tag="z")
            nc.vector.tensor_add(z_cur[:], et[:], mu_sb[:])

            # ---- C = WiC.T @ catC ----
            cpp = decp.tile([128, 1024], F32, name="cpp", tag="dec")
            nc.tensor.matmul(out=cpp[:, 0:512], lhsT=s_wica[:], rhs=s_catCA[:],
                             start=True, stop=False)
            nc.tensor.matmul(out=cpp[:, 0:512], lhsT=s_wicb[:], rhs=s_catCB[:],
                             start=False, stop=True)
            s_C = cp.tile([128, BS], F32, name="s_C", tag="s_C")
            nc.vector.tensor_copy(s_C[:], cpp[:, 0:512])

            # ---- decoder emitter ----
            def emit_decode(z_tile, t):
                for c in range(4):
                    dp = decp.tile([128, 1024], F32, name=f"dp{t}_{c}", tag="dec")
                    zch = z_tile[:, c * 128:(c + 1) * 128]
                    nc.tensor.matmul(out=dp[:, 0:512], lhsT=zch, rhs=s_decwa[:],
                                     start=True, stop=True)
                    nc.tensor.matmul(out=dp[:, 512:1024], lhsT=zch, rhs=s_decwb[:],
                                     start=True, stop=True)
                    stg = sp.tile([128, 1024], F32, name=f"stg{t}_{c}", tag="stg")
                    nc.vector.tensor_add(stg[:], dp[:], s_decbb[:])
                    dst = logits.ap()[c * 128:(c + 1) * 128, t:t + 1, :].rearrange(
                        "p a (h z) -> p (a h) z", h=2)
                    src = stg[:].rearrange("p (h z) -> p h z", z=512)[:, :, 0:500]
                    nc.sync.dma_start(dst, src)

            emit_decode(z_cur, 0)

            # ---- RK4 steps ----
            import concourse.alu_op_type as alu
            ADD = alu.AluOpType.add
            Dp = dpp.tile([128, BS], F32, name="Dp0", tag="Dp", bufs=2)
            nc.tensor.matmul(out=Dp[:], lhsT=s_i128[:], rhs=s_C[:],
                             start=True, stop=False)
            nc.tensor.matmul(out=Dp[:], lhsT=s_wiz[:], rhs=z_cur[:],
                             start=False, stop=True)
            for s in range(NSTEP):
                hfin = [[None] * 4 for _ in range(NST)]
                # start next step's base D early: C and z_s terms have no
                # dependency on this step's work; hh terms accumulate below.
                if s < NSTEP - 1:
                    Dn = dpp.tile([128, BS], F32, name=f"Dn{s}", tag="Dp", bufs=2)
                    nc.tensor.matmul(out=Dn[:], lhsT=s_i128[:], rhs=s_C[:],
                                     start=True, stop=False)
                    nc.tensor.matmul(out=Dn[:], lhsT=s_wiz[:], rhs=z_cur[:],
                                     start=False, stop=False)
                Dsb = wp.tile([128, BS], F32, name=f"Dsb{s}", tag="Dsb", bufs=2)
                nc.vector.tensor_copy(Dsb[:], Dp[:])
                for e in range(4):
                    bidx = 0 if e == 0 else (1 if e < 3 else 2)
                    bias_ap = s_btab[:, s * 3 + bidx:s * 3 + bidx + 1]
                    wf = None if e == 0 else (s_wfh if e < 3 else s_wff)
                    for st in range(NST):
                        cs = slice(st * FD, (st + 1) * FD)
                        if wf is None:
                            xp_ap = Dp[:, cs]
                        else:
                            xp = xpp.tile([128, FD], F32, name=f"xp{s}_{e}_{st}",
                                          tag="xp")
                            nc.tensor.matmul(out=xp[:], lhsT=s_i128[:],
                                             rhs=Dsb[:, cs], start=True, stop=False)
                            nc.tensor.matmul(out=xp[:], lhsT=wf[:],
                                             rhs=hfin[st][e - 1][:],
                                             start=False, stop=True)
                            xp_ap = xp[:]
                        hh = wp.tile([128, FD], F32, name=f"hh{s}_{e}_{st}",
                                     tag=f"tmp{st}")
                        nc.scalar.activation(hh[:], xp_ap, AF.Tanh, bias=bias_ap)
                        for rbi in range(2):
                            wa, wb = s_rb[2 * rbi], s_rb[2 * rbi + 1]
                            rp = xpp.tile([128, FD], F32, name=f"rp{s}_{e}_{st}_{rbi}",
                                          tag="xp")
                            nc.tensor.matmul(out=rp[:], lhsT=wa[:], rhs=hh[:],
                                             start=True, stop=True)
                            t1 = wp.tile([128, FD], F32, name=f"t1{s}_{e}_{st}_{rbi}",
                                         tag=f"t1_{st}")
                            nc.scalar.activation(t1[:], rp[:], AF.Tanh,
                                                 bias=s_rbb[:, 2 * rbi:2 * rbi + 1])
                            rp2 = xpp.tile([128, FD], F32, name=f"rq{s}_{e}_{st}_{rbi}",
                                           tag="xp")
                            if SKIP_PE:
                                # skip connection on the PE: one fewer
                                # cross-engine hop on the serial chain
                                nc.tensor.matmul(out=rp2[:], lhsT=wb[:], rhs=t1[:],
                                                 start=True, stop=False)
                                nc.tensor.matmul(out=rp2[:], lhsT=s_i128[:],
                                                 rhs=hh[:], start=False, stop=True)
                            else:
                                nc.tensor.matmul(out=rp2[:], lhsT=wb[:], rhs=t1[:],
                                                 start=True, stop=True)
                                nc.vector.tensor_add(rp2[:], rp2[:], hh[:])
                            is_final = rbi == 1
                            hh = wp.tile([128, FD], F32,
                                         name=f"ho{s}_{e}_{st}_{rbi}",
                                         tag=(f"hfin{st}" if is_final else f"tmp{st}"),
                                         bufs=(6 if is_final else None))
                            nc.scalar.activation(hh[:], rp2[:], AF.Tanh,
                                                 bias=s_rbb[:, 2 * rbi + 1:2 * rbi + 2])
                        hfin[st][e] = hh
                        if s < NSTEP - 1:
                            wzn = s_wz6 if e in (0, 3) else s_wz3
                            nc.tensor.matmul(out=Dn[:, cs], lhsT=wzn[:], rhs=hh[:],
                                             start=False, stop=(e == 3))
                # z update: PSUM accumulates the four Wo terms; the z/bo adds
                # ride the PSUM->SBUF evacuation (one fused DVE op).
                znp = znpp.tile([L, BS], F32, name=f"znp{s}", tag="znp")
                for st in range(NST):
                    cs = slice(st * FD, (st + 1) * FD)
                    nc.tensor.matmul(out=znp[:, cs], lhsT=s_wo6[:], rhs=hfin[st][0][:],
                                     start=True, stop=False)
                    nc.tensor.matmul(out=znp[:, cs], lhsT=s_wo3[:], rhs=hfin[st][1][:],
                                     start=False, stop=False)
                    nc.tensor.matmul(out=znp[:, cs], lhsT=s_wo3[:], rhs=hfin[st][2][:],
                                     start=False, stop=False)
                    nc.tensor.matmul(out=znp[:, cs], lhsT=s_wo6[:], rhs=hfin[st][3][:],
                                     start=False, stop=True)
                z_next = zp.tile([L, BS], F32, name=f"z{s + 1}", tag="z")
                nc.vector.scalar_tensor_tensor(
                    out=z_next[:], in0=znp[:], scalar=s_bov[:, 0:1], in1=z_cur[:],
                    op0=ADD, op1=ADD)
                z_cur = z_next
                if s < NSTEP - 1:
                    Dp = Dn
                emit_decode(z_cur, s + 1)

    _split_excess_waits(nc, max_waits=1)
    return nc


def _host_prep(inputs):
    """Returns (in_maps list per core, dt)."""
    f32 = np.float32
    times = np.asarray(inputs["times"], f32)
    dts = np.diff(times)
    dt = float(dts[0])
    assert np.allclose(dts, dt, atol=1e-6), "non-uniform times unsupported"

    zt = np.asarray(inputs["zone_table"], f32)
    person = np.asarray(inputs["person_features"], f32)
    purpose = np.asarray(inputs["purpose_features"], f32)
    eps = np.asarray(inputs["eps"], f32)
    hid = np.asarray(inputs["home_zone_id"]).astype(np.int64)
    wid = np.asarray(inputs["work_zone_id"]).astype(np.int64)
    home = zt[hid]
    work = zt[wid]

    Wi = np.asarray(inputs["ode_Wi"], f32)
    Wo = np.asarray(inputs["ode_Wo"], f32)
    bi = np.asarray(inputs["ode_bi"], f32)
    bo = np.asarray(inputs["ode_bo"], f32)
    Wiz = Wi[0:64]
    Wit = Wi[64]
    WiC = np.concatenate([Wi[65:97], Wi[97:161], Wi[161:225]], 0)
    bfuse = bo @ Wiz
    btab = np.zeros((128, NSTEP * 3 + VARIANT), f32)
    for s in range(NSTEP):
        t0 = float(times[s])
        drift = (dt * bfuse) if s >= 1 else 0.0
        btab[:, s * 3 + 0] = bi + t0 * Wit + drift
        btab[:, s * 3 + 1] = bi + (t0 + dt / 2) * Wit + (dt / 2) * bfuse + drift
        btab[:, s * 3 + 2] = bi + (t0 + dt) * Wit + dt * bfuse + drift

    eW1 = np.asarray(inputs["enc_W1"], f32)
    eb1 = np.asarray(inputs["enc_b1"], f32)
    eW2 = np.asarray(inputs["enc_W2"], f32)
    eb2 = np.asarray(inputs["enc_b2"], f32)
    decW = np.asarray(inputs["dec_W"], f32)
    decb = np.asarray(inputs["dec_b"], f32)
    decbb = np.zeros((128, 1024), f32)
    decbb[:, 0:500] = decb[0:500]
    decbb[:, 512:1012] = decb[500:1000]

    shared = {
        "btab": np.ascontiguousarray(btab),
        "w1a": np.ascontiguousarray(eW1[0:128]),
        "w1b": np.ascontiguousarray(eW1[128:168]),
        "b1c": np.ascontiguousarray(np.stack([eb1[0:128], eb1[128:256]], 1)),
        "w2": np.ascontiguousarray(eW2),
        "b2mu": np.ascontiguousarray(eb2[0:64, None]),
        "b2lvh": np.ascontiguousarray(0.5 * eb2[64:128, None]),
        "wica": np.ascontiguousarray(WiC[0:128]),
        "wicb": np.ascontiguousarray(WiC[128:160]),
        "i128": np.eye(128, dtype=f32),
        "i64": np.eye(64, dtype=f32),
        "wiz": np.ascontiguousarray(Wiz),
        "wfh": np.ascontiguousarray((dt / 2) * (Wo @ Wiz)),
        "wff": np.ascontiguousarray(dt * (Wo @ Wiz)),
        "wz6": np.ascontiguousarray((dt / 6) * (Wo @ Wiz)),
        "wz3": np.ascontiguousarray((dt / 3) * (Wo @ Wiz)),
        "rb1w1": np.asarray(inputs["rb1_W1"], f32),
        "rb1w2": np.asarray(inputs["rb1_W2"], f32),
        "rb2w1": np.asarray(inputs["rb2_W1"], f32),
        "rb2w2": np.asarray(inputs["rb2_W2"], f32),
        "rbb": np.ascontiguousarray(np.stack(
            [np.asarray(inputs["rb1_b1"], f32), np.asarray(inputs["rb1_b2"], f32),
             np.asarray(inputs["rb2_b1"], f32), np.asarray(inputs["rb2_b2"], f32)], 1)),
        "wo6": np.ascontiguousarray((dt / 6) * Wo),
        "wo3": np.ascontiguousarray((dt / 3) * Wo),
        "bov": np.ascontiguousarray(dt * bo[:, None]),
        "decwa": np.concatenate([decW[:, 0:500], np.zeros((64, 12), f32)], 1),
        "decwb": np.concatenate([decW[:, 500:1000], np.zeros((64, 12), f32)], 1),
        "decbb": decbb,
    }

    in_maps = []
    for c in range(N_CORES):
        sl = slice(c * BS, (c + 1) * BS)
        pT = np.ascontiguousarray(person[sl].T)
        hT = np.ascontiguousarray(home[sl].T)
        wT = np.ascontiguousarray(work[sl].T)
        gT = np.ascontiguousarray(purpose[sl].T)
        encCat = np.concatenate([pT, hT, wT, gT], 0)        # [168, BS]
        catC = np.concatenate([pT, wT, hT], 0)              # [160, BS]
        in_maps.append({
            "encCatA": np.ascontiguousarray(encCat[0:128]),
            "encCatB": np.ascontiguousarray(encCat[128:168]),
            "catCA": np.ascontiguousarray(catC[0:128]),
            "catCB": np.ascontiguousarray(catC[128:160]),
            "epsT": np.ascontiguousarray(eps[sl].T),
            **shared,
        })
    return in_maps


_cached = {}


def _get_runner():
    if "runner" not in _cached:
        import jax
        import jax.numpy as jnp
        from jax.sharding import Mesh, PartitionSpec
        from jax.experimental.shard_map import shard_map
        from concourse import bass2jax

        nc = _build_program()
        bass2jax.install_neuronx_cc_hook()
        partition_name = nc.partition_id_tensor.name if nc.partition_id_tensor else None
        in_names, out_names, out_avals = [], [], []
        for alloc in nc.m.functions[0].allocations:
            if not isinstance(alloc, mybir.MemoryLocationSet):
                continue
            name = alloc.memorylocations[0].name
            if alloc.kind == "ExternalInput":
                if name != partition_name:
                    in_names.append(name)
            elif alloc.kind == "ExternalOutput":
                out_names.append(name)
                out_avals.append(jax.core.ShapedArray(
                    tuple(alloc.tensor_shape), mybir.dt.np(alloc.dtype)))
        n_params, n_outs = len(in_names), len(out_names)
        all_in_names = list(in_names) + list(out_names)
        if partition_name is not None:
            all_in_names.append(partition_name)
        devices = jax.devices()[:N_CORES]
        mesh = Mesh(np.asarray(devices), ("core",))

        def _body(*args):
            operands = list(args)
            if partition_name is not None:
                operands.append(bass2jax.partition_id_tensor())
            outs = bass2jax._bass_exec_p.bind(
                *operands,
                out_avals=tuple(out_avals),
                in_names=tuple(all_in_names),
                out_names=tuple(out_names),
                lowering_input_output_aliases=(),
                sim_require_finite=True,
                sim_require_nnan=True,
                nc=nc,
            )
            return tuple(outs)

        fn = jax.jit(shard_map(
            _body, mesh=mesh,
            in_specs=(PartitionSpec("core"),) * (n_params + n_outs),
            out_specs=(PartitionSpec("core"),) * n_outs,
            check_rep=False,
        ))
        sharding = jax.sharding.NamedSharding(mesh, PartitionSpec("core"))
        zmaker = jax.jit(
            lambda: tuple(
                jnp.zeros((N_CORES * av.shape[0], *av.shape[1:]), av.dtype)
                for av in out_avals),
            out_shardings=tuple(sharding for _ in out_avals),
        )
        zeros = zmaker()
        jax.block_until_ready(zeros)
        _cached["runner"] = (fn, zeros, in_names, out_names, out_avals, mesh)
    return _cached["runner"]


def _put_inputs(in_maps):
    import jax
    from jax.sharding import PartitionSpec
    fn, zeros, in_names, out_names, out_avals, mesh = _get_runner()
    sharding = jax.sharding.NamedSharding(mesh, PartitionSpec("core"))
    concat = [
        np.concatenate([np.asarray(m[name]) for m in in_maps], axis=0)
        for name in in_names
    ]
    return [jax.device_put(a, sharding) for a in concat]


def _run_device(dev_in):
    import jax
    fn, zeros, in_names, out_names, out_avals, mesh = _get_runner()
    outs = fn(*dev_in, *zeros)
    jax.block_until_ready(outs)
    return outs


def kernel(**inputs):
    in_maps = _host_prep(inputs)
    last_err = None
    for attempt in range(3):
        try:
            dev_in = _put_inputs(in_maps)
            outs = _run_device(dev_in)
            break
        except Exception as e:  # device can wedge transiently; rebuild + retry
            last_err = e
            _cached.clear()
            import time as _time
            _time.sleep(20 * (attempt + 1))
    else:
        raise last_err
    fn, zeros, in_names, out_names, out_avals, mesh = _get_runner()
    host = {name: np.asarray(o) for name, o in zip(out_names, outs)}
    logits = host["logits"].reshape(N_CORES, BS, T, Z).reshape(B, T, Z)
    muT = host["muT"].reshape(N_CORES, L, BS)
    lvT = host["lvT"].reshape(N_CORES, L, BS)
    mu = np.concatenate([muT[c].T for c in range(N_CORES)], 0)
    lv = np.concatenate([lvT[c].T for c in range(N_CORES)], 0)
    return (logits.astype(np.float32), mu.astype(np.float32), lv.astype(np.float32))


# revision 9
# speedup vs baseline: 1.2122x; 1.2122x over previous
"""Trainium2 Bass kernel for nn_GenerativeODE (VAE encoder + RK4 neural-ODE +
zone decoder), data-parallel over 8 NeuronCores (batch 4096 -> 512/core).

Layout: feature-major on device ([feature, sample]); batch 512/core processed
as 2 interleaved substreams of 256 so ScalarE (tanh) and TensorE overlap.

Key restructuring (validated in numpy to 8e-7 rel err):
  - ode_Wo is folded into the next RK4 stage's input matmul:
      Wi_z.T @ (z + a*k_prev) = Wi_z.T @ z + a*(Wo@Wi_z).T @ hh_prev + const
    so k tensors are never materialized; everything accumulates in PSUM.
  - time/bias terms are per-(step, stage) per-partition bias vectors applied
    for free by the ACTIVATE instruction (tanh(scale*x + bias)).
  - z_{t+1} = z_t + Wo6.T@hh1 + Wo3.T@hh2 + Wo3.T@hh3 + Wo6.T@hh4 (+dt*bo)
    accumulated on the PE via an identity matmul for the z term.
  - decoder runs batch-major (lhsT = z chunk) so logits DMA out is contiguous
    4KB rows; overlaps the next ODE step.
"""
import os
import sys
import numpy as np

for _p in ("/opt/trn_rl_repo", "/root/.axon_site/_ro/trn_rl_repo"):
    if os.path.isdir(_p) and _p not in sys.path:
        sys.path.append(_p)

import concourse.bass as bass
import concourse.mybir as mybir
import concourse.tile as tile

F32 = mybir.dt.float32
AF = mybir.ActivationFunctionType

N_CORES = 8
B, P, Z, ZE, PG, EH, L, OH, T = 4096, 32, 1000, 64, 8, 256, 64, 128, 48
BS = B // N_CORES          # 512 per core
NST = int(os.environ.get("KNST", "1"))  # substreams
SKIP_PE = os.environ.get("KSKIP", "pe") == "pe"  # resblock skip-add engine
VARIANT = NST + (8 if SKIP_PE else 0)
FD = BS // NST             # 256
NSTEP = T - 1              # 47

_wsplit_counter = [0]


def _split_excess_waits(nc, max_waits=1):
    """walrus CoreV2/V3 codegen allows only one sync-wait per instruction;
    split extra waits onto NoOps inserted before the offending instruction."""
    for fn in nc.m.functions:
        for bb in fn.blocks:
            insts = bb.instructions
            if not any(
                getattr(i, "sync_info", None) is not None
                and i.sync_info is not None
                and len(i.sync_info.on_wait) > max_waits
                for i in insts
            ):
                continue
            new = []
            for ins in insts:
                si = getattr(ins, "sync_info", None)
                if si is not None and len(si.on_wait) > max_waits:
                    waits = list(si.on_wait)
                    while len(waits) > max_waits:
                        chunk, waits = waits[:max_waits], waits[max_waits:]
                        _wsplit_counter[0] += 1
                        nop = mybir.InstNoOp(name=f"I-wsplit-{_wsplit_counter[0]}")
                        nop.engine = ins.engine
                        nop.sync_info = mybir.SyncInfo(on_wait=chunk, on_update=[])
                        new.append(nop)
                    ins.sync_info = mybir.SyncInfo(
                        on_wait=waits, on_update=list(si.on_update)
                    )
                new.append(ins)
            bb.instructions = new


def _build_program():
    nc = bass.Bass("TRN2", target_bir_lowering=False, debug=False)
    D = lambda name, shape: nc.dram_tensor(name, list(shape), F32, kind="ExternalInput")
    # per-core tensors
    encCatA = D("encCatA", (128, BS))
    encCatB = D("encCatB", (40, BS))
    catCA = D("catCA", (128, BS))
    catCB = D("catCB", (32, BS))
    epsT = D("epsT", (L, BS))
    btab = D("btab", (128, NSTEP * 3 + VARIANT))
    # replicated weights
    w1a = D("w1a", (128, EH))
    w1b = D("w1b", (40, EH))
    b1c = D("b1c", (128, 2))          # enc_b1 as two [128,1] columns
    w2 = D("w2", (EH, 128))
    b2mu = D("b2mu", (L, 1))
    b2lvh = D("b2lvh", (L, 1))
    wica = D("wica", (128, OH))
    wicb = D("wicb", (32, OH))
    i128 = D("i128", (128, 128))
    i64 = D("i64", (L, L))
    wiz = D("wiz", (L, OH))
    wfh = D("wfh", (OH, OH))
    wff = D("wff", (OH, OH))
    rb1w1 = D("rb1w1", (OH, OH))
    rb1w2 = D("rb1w2", (OH, OH))
    rb2w1 = D("rb2w1", (OH, OH))
    rb2w2 = D("rb2w2", (OH, OH))
    rbb = D("rbb", (OH, 4))
    wo6 = D("wo6", (OH, L))
    wo3 = D("wo3", (OH, L))
    bov = D("bov", (L, 1))
    decwa = D("decwa", (L, 512))
    decwb = D("decwb", (L, 512))
    decbb = D("decbb", (128, 1024))

    logits = nc.dram_tensor("logits", [BS, T, Z], F32, kind="ExternalOutput")
    muT = nc.dram_tensor("muT", [L, BS], F32, kind="ExternalOutput")
    lvT = nc.dram_tensor("lvT", [L, BS], F32, kind="ExternalOutput")

    with tile.TileContext(nc) as tc:
        with tc.tile_pool(name="const", bufs=1) as cp, \
             tc.tile_pool(name="zpool", bufs=3) as zp, \
             tc.tile_pool(name="work", bufs=4) as wp, \
             tc.tile_pool(name="stage", bufs=3) as sp, \
             tc.tile_pool(name="xps", bufs=(2 if NST == 1 else 3), space="PSUM") as xpp, \
             tc.tile_pool(name="znps", bufs=1, space="PSUM") as znpp, \
             tc.tile_pool(name="dps", bufs=(1 if NST == 1 else 2), space="PSUM") as dpp, \
             tc.tile_pool(name="decps", bufs=(2 if NST == 1 else 1), space="PSUM") as decp:

            # ---- load constants into SBUF ----
            def ld(dram, shape, name):
                t_ = cp.tile(list(shape), F32, name=name, tag=name)
                nc.sync.dma_start(t_[:], dram.ap())
                return t_

            s_encCatA = ld(encCatA, (128, BS), "s_encCatA")
            s_encCatB = ld(encCatB, (40, BS), "s_encCatB")
            s_catCA = ld(catCA, (128, BS), "s_catCA")
            s_catCB = ld(catCB, (32, BS), "s_catCB")
            s_epsT = ld(epsT, (L, BS), "s_epsT")
            s_btab = ld(btab, (128, NSTEP * 3 + VARIANT), "s_btab")
            s_w1a = ld(w1a, (128, EH), "s_w1a")
            s_w1b = ld(w1b, (40, EH), "s_w1b")
            s_b1c = ld(b1c, (128, 2), "s_b1c")
            # enc_W2 is [256,128]: SBUF tiles cap at 128 partitions, so load
            # it as two K-chunks.
            s_w2 = cp.tile([128, 128], F32, name="s_w2", tag="s_w2")
            nc.sync.dma_start(s_w2[:], w2.ap()[0:128, :])
            s_b2mu = ld(b2mu, (L, 1), "s_b2mu")
            s_b2lvh = ld(b2lvh, (L, 1), "s_b2lvh")
            s_wica = ld(wica, (128, OH), "s_wica")
            s_wicb = ld(wicb, (32, OH), "s_wicb")
            s_i128 = ld(i128, (128, 128), "s_i128")
            s_i64 = ld(i64, (L, L), "s_i64")
            s_wiz = ld(wiz, (L, OH), "s_wiz")
            s_wfh = ld(wfh, (OH, OH), "s_wfh")
            s_wff = ld(wff, (OH, OH), "s_wff")
            s_rb = [
                ld(rb1w1, (OH, OH), "s_rb1w1"),
                ld(rb1w2, (OH, OH), "s_rb1w2"),
                ld(rb2w1, (OH, OH), "s_rb2w1"),
                ld(rb2w2, (OH, OH), "s_rb2w2"),
            ]
            s_rbb = ld(rbb, (OH, 4), "s_rbb")
            s_wo6 = ld(wo6, (OH, L), "s_wo6")
            s_wo3 = ld(wo3, (OH, L), "s_wo3")
            s_bov = ld(bov, (L, 1), "s_bov")
            s_decwa = ld(decwa, (L, 512), "s_decwa")
            s_decwb = ld(decwb, (L, 512), "s_decwb")
            s_decbb = ld(decbb, (128, 1024), "s_decbb")

            # ---- encoder ----
            hs = []
            for m in range(2):
                hp = decp.tile([128, 1024], F32, name=f"hp{m}", tag="dec")
                nc.tensor.matmul(out=hp[:, 0:512], lhsT=s_w1a[:, m * 128:(m + 1) * 128],
                                 rhs=s_encCatA[:], start=True, stop=False)
                nc.tensor.matmul(out=hp[:, 0:512], lhsT=s_w1b[:, m * 128:(m + 1) * 128],
                                 rhs=s_encCatB[:], start=False, stop=True)
                h = wp.tile([128, BS], F32, name=f"h{m}", tag=f"ench{m}")
                nc.scalar.activation(h[:], hp[:, 0:512], AF.Relu, bias=s_b1c[:, m:m + 1])
                hs.append(h)
            lp = decp.tile([128, 1024], F32, name="lp", tag="dec")
            nc.tensor.matmul(out=lp[:, 0:512], lhsT=s_w2[:], rhs=hs[0][:],
                             start=True, stop=False)
            # second K-chunk of enc_W2 lives in rows 128:256 -> separate tile
            # (SBUF tiles are <=128 partitions, so w2 is passed as [256,128]
            # DRAM and loaded into two tiles)
            s_w2b = cp.tile([128, 128], F32, name="s_w2b", tag="s_w2b")
            nc.sync.dma_start(s_w2b[:], w2.ap()[128:256, :])
            nc.tensor.matmul(out=lp[:, 0:512], lhsT=s_w2b[:], rhs=hs[1][:],
                             start=False, stop=True)

            mu_sb = wp.tile([L, BS], F32, name="mu_sb", tag="mu_sb")
            nc.vector.tensor_scalar_add(mu_sb[:], lp[0:64, 0:512], s_b2mu[:, 0:1])
            nc.sync.dma_start(muT.ap(), mu_sb[:])
            lv_sb = wp.tile([L, BS], F32, name="lv_sb", tag="lv_sb")
            nc.vector.tensor_scalar_add(lv_sb[:], lp[64:128, 0:512], s_b2lvh[:, 0:1])
            # lvT output: b2lvh holds 0.5*b2lv, so add it twice for the raw
            # log_var output (lv = lp + b2lv = lp + 2*(0.5 b2lv)).
            lv_out = wp.tile([L, BS], F32, name="lv_out", tag="lv_out")
            nc.vector.tensor_scalar_add(lv_out[:], lv_sb[:], s_b2lvh[:, 0:1])
            nc.sync.dma_start(lvT.ap(), lv_out[:])
            std_sb = wp.tile([L, BS], F32, name="std_sb", tag="std_sb")
            nc.scalar.activation(std_sb[:], lp[64:128, 0:512], AF.Exp,
                                 bias=s_b2lvh[:, 0:1], scale=0.5)
            et = wp.tile([L, BS], F32, name="et", tag="et")
            nc.vector.tensor_mul(et[:], s_epsT[:], std_sb[:])
            z_cur = zp.tile([L, BS], F32, name="z0", tag="z")
            nc.vector.tensor_add(z_cur[:], et[:], mu_sb[:])

            # ---- C = WiC.T @ catC ----
            cpp = decp.tile([128, 1024], F32, name="cpp", tag="dec")
            nc.tensor.matmul(out=cpp[:, 0:512], lhsT=s_wica[:], rhs=s_catCA[:],
                             start=True, stop=False)
            nc.tensor.matmul(out=cpp[:, 0:512], lhsT=s_wicb[:], rhs=s_catCB[:],
                             start=False, stop=True)
            s_C = cp.tile([128, BS], F32, name="s_C", tag="s_C")
            nc.vector.tensor_copy(s_C[:], cpp[:, 0:512])

            # ---- decoder emitter ----
            def emit_decode(z_tile, t):
                for c in range(4):
                    dp = decp.tile([128, 1024], F32, name=f"dp{t}_{c}", tag="dec")
                    zch = z_tile[:, c * 128:(c + 1) * 128]
                    nc.tensor.matmul(out=dp[:, 0:512], lhsT=zch, rhs=s_decwa[:],
                                     start=True, stop=True)
                    nc.tensor.matmul(out=dp[:, 512:1024], lhsT=zch, rhs=s_decwb[:],
                                     start=True, stop=True)
                    stg = sp.tile([128, 1024], F32, name=f"stg{t}_{c}", tag="stg")
                    nc.vector.tensor_add(stg[:], dp[:], s_decbb[:])
                    dst = logits.ap()[c * 128:(c + 1) * 128, t:t + 1, :].rearrange(
                        "p a (h z) -> p (a h) z", h=2)
                    src = stg[:].rearrange("p (h z) -> p h z", z=512)[:, :, 0:500]
                    nc.sync.dma_start(dst, src)

            emit_decode(z_cur, 0)

            # ---- RK4 steps ----
            import concourse.alu_op_type as alu
            ADD = alu.AluOpType.add
            for s in range(NSTEP):
                hfin = [[None] * 4 for _ in range(NST)]
                # per-step base D = C + Wiz.T @ z (one bank, both streams)
                Dp = dpp.tile([128, BS], F32, name=f"Dp{s}", tag="Dp")
                nc.tensor.matmul(out=Dp[:], lhsT=s_i128[:], rhs=s_C[:],
                                 start=True, stop=False)
                nc.tensor.matmul(out=Dp[:], lhsT=s_wiz[:], rhs=z_cur[:],
                                 start=False, stop=True)
                Dsb = wp.tile([128, BS], F32, name=f"Dsb{s}", tag="Dsb", bufs=2)
                nc.vector.tensor_copy(Dsb[:], Dp[:])
                for e in range(4):
                    bidx = 0 if e == 0 else (1 if e < 3 else 2)
                    bias_ap = s_btab[:, s * 3 + bidx:s * 3 + bidx + 1]
                    wf = None if e == 0 else (s_wfh if e < 3 else s_wff)
                    for st in range(NST):
                        cs = slice(st * FD, (st + 1) * FD)
                        if wf is None:
                            xp_ap = Dp[:, cs]
                        else:
                            xp = xpp.tile([128, FD], F32, name=f"xp{s}_{e}_{st}",
                                          tag="xp")
                            nc.tensor.matmul(out=xp[:], lhsT=s_i128[:],
                                             rhs=Dsb[:, cs], start=True, stop=False)
                            nc.tensor.matmul(out=xp[:], lhsT=wf[:],
                                             rhs=hfin[st][e - 1][:],
                                             start=False, stop=True)
                            xp_ap = xp[:]
                        hh = wp.tile([128, FD], F32, name=f"hh{s}_{e}_{st}",
                                     tag=f"tmp{st}")
                        nc.scalar.activation(hh[:], xp_ap, AF.Tanh, bias=bias_ap)
                        for rbi in range(2):
                            wa, wb = s_rb[2 * rbi], s_rb[2 * rbi + 1]
                            rp = xpp.tile([128, FD], F32, name=f"rp{s}_{e}_{st}_{rbi}",
                                          tag="xp")
                            nc.tensor.matmul(out=rp[:], lhsT=wa[:], rhs=hh[:],
                                             start=True, stop=True)
                            t1 = wp.tile([128, FD], F32, name=f"t1{s}_{e}_{st}_{rbi}",
                                         tag=f"t1_{st}")
                            nc.scalar.activation(t1[:], rp[:], AF.Tanh,
                                                 bias=s_rbb[:, 2 * rbi:2 * rbi + 1])
                            rp2 = xpp.tile([128, FD], F32, name=f"rq{s}_{e}_{st}_{rbi}",
                                           tag="xp")
                            if SKIP_PE:
                                # skip connection on the PE: one fewer
                                # cross-engine hop on the serial chain
                                nc.tensor.matmul(out=rp2[:], lhsT=wb[:], rhs=t1[:],
                                                 start=True, stop=False)
                                nc.tensor.matmul(out=rp2[:], lhsT=s_i128[:],
                                                 rhs=hh[:], start=False, stop=True)
                            else:
                                nc.tensor.matmul(out=rp2[:], lhsT=wb[:], rhs=t1[:],
                                                 start=True, stop=True)
                                nc.vector.tensor_add(rp2[:], rp2[:], hh[:])
                            is_final = rbi == 1
                            hh = wp.tile([128, FD], F32,
                                         name=f"ho{s}_{e}_{st}_{rbi}",
                                         tag=(f"hfin{st}" if is_final else f"tmp{st}"),
                                         bufs=(6 if is_final else None))
                            nc.scalar.activation(hh[:], rp2[:], AF.Tanh,
                                                 bias=s_rbb[:, 2 * rbi + 1:2 * rbi + 2])
                        hfin[st][e] = hh
                # z update: PSUM accumulates the four Wo terms; the z/bo adds
                # ride the PSUM->SBUF evacuation (one fused DVE op).
                znp = znpp.tile([L, BS], F32, name=f"znp{s}", tag="znp")
                for st in range(NST):
                    cs = slice(st * FD, (st + 1) * FD)
                    nc.tensor.matmul(out=znp[:, cs], lhsT=s_wo6[:], rhs=hfin[st][0][:],
                                     start=True, stop=False)
                    nc.tensor.matmul(out=znp[:, cs], lhsT=s_wo3[:], rhs=hfin[st][1][:],
                                     start=False, stop=False)
                    nc.tensor.matmul(out=znp[:, cs], lhsT=s_wo3[:], rhs=hfin[st][2][:],
                                     start=False, stop=False)
                    nc.tensor.matmul(out=znp[:, cs], lhsT=s_wo6[:], rhs=hfin[st][3][:],
                                     start=False, stop=True)
                z_next = zp.tile([L, BS], F32, name=f"z{s + 1}", tag="z")
                nc.vector.scalar_tensor_tensor(
                    out=z_next[:], in0=znp[:], scalar=s_bov[:, 0:1], in1=z_cur[:],
                    op0=ADD, op1=ADD)
                z_cur = z_next
                emit_decode(z_cur, s + 1)

    _split_excess_waits(nc, max_waits=1)
    return nc


def _host_prep(inputs):
    """Returns (in_maps list per core, dt)."""
    f32 = np.float32
    times = np.asarray(inputs["times"], f32)
    dts = np.diff(times)
    dt = float(dts[0])
    assert np.allclose(dts, dt, atol=1e-6), "non-uniform times unsupported"

    zt = np.asarray(inputs["zone_table"], f32)
    person = np.asarray(inputs["person_features"], f32)
    purpose = np.asarray(inputs["purpose_features"], f32)
    eps = np.asarray(inputs["eps"], f32)
    hid = np.asarray(inputs["home_zone_id"]).astype(np.int64)
    wid = np.asarray(inputs["work_zone_id"]).astype(np.int64)
    home = zt[hid]
    work = zt[wid]

    Wi = np.asarray(inputs["ode_Wi"], f32)
    Wo = np.asarray(inputs["ode_Wo"], f32)
    bi = np.asarray(inputs["ode_bi"], f32)
    bo = np.asarray(inputs["ode_bo"], f32)
    Wiz = Wi[0:64]
    Wit = Wi[64]
    WiC = np.concatenate([Wi[65:97], Wi[97:161], Wi[161:225]], 0)
    bfuse = bo @ Wiz
    btab = np.zeros((128, NSTEP * 3 + VARIANT), f32)
    for s in range(NSTEP):
        t0 = float(times[s])
        btab[:, s * 3 + 0] = bi + t0 * Wit
        btab[:, s * 3 + 1] = bi + (t0 + dt / 2) * Wit + (dt / 2) * bfuse
        btab[:, s * 3 + 2] = bi + (t0 + dt) * Wit + dt * bfuse

    eW1 = np.asarray(inputs["enc_W1"], f32)
    eb1 = np.asarray(inputs["enc_b1"], f32)
    eW2 = np.asarray(inputs["enc_W2"], f32)
    eb2 = np.asarray(inputs["enc_b2"], f32)
    decW = np.asarray(inputs["dec_W"], f32)
    decb = np.asarray(inputs["dec_b"], f32)
    decbb = np.zeros((128, 1024), f32)
    decbb[:, 0:500] = decb[0:500]
    decbb[:, 512:1012] = decb[500:1000]

    shared = {
        "btab": np.ascontiguousarray(btab),
        "w1a": np.ascontiguousarray(eW1[0:128]),
        "w1b": np.ascontiguousarray(eW1[128:168]),
        "b1c": np.ascontiguousarray(np.stack([eb1[0:128], eb1[128:256]], 1)),
        "w2": np.ascontiguousarray(eW2),
        "b2mu": np.ascontiguousarray(eb2[0:64, None]),
        "b2lvh": np.ascontiguousarray(0.5 * eb2[64:128, None]),
        "wica": np.ascontiguousarray(WiC[0:128]),
        "wicb": np.ascontiguousarray(WiC[128:160]),
        "i128": np.eye(128, dtype=f32),
        "i64": np.eye(64, dtype=f32),
        "wiz": np.ascontiguousarray(Wiz),
        "wfh": np.ascontiguousarray((dt / 2) * (Wo @ Wiz)),
        "wff": np.ascontiguousarray(dt * (Wo @ Wiz)),
        "rb1w1": np.asarray(inputs["rb1_W1"], f32),
        "rb1w2": np.asarray(inputs["rb1_W2"], f32),
        "rb2w1": np.asarray(inputs["rb2_W1"], f32),
        "rb2w2": np.asarray(inputs["rb2_W2"], f32),
        "rbb": np.ascontiguousarray(np.stack(
            [np.asarray(inputs["rb1_b1"], f32), np.asarray(inputs["rb1_b2"], f32),
             np.asarray(inputs["rb2_b1"], f32), np.asarray(inputs["rb2_b2"], f32)], 1)),
        "wo6": np.ascontiguousarray((dt / 6) * Wo),
        "wo3": np.ascontiguousarray((dt / 3) * Wo),
        "bov": np.ascontiguousarray(dt * bo[:, None]),
        "decwa": np.concatenate([decW[:, 0:500], np.zeros((64, 12), f32)], 1),
        "decwb": np.concatenate([decW[:, 500:1000], np.zeros((64, 12), f32)], 1),
        "decbb": decbb,
    }

    in_maps = []
    for c in range(N_CORES):
        sl = slice(c * BS, (c + 1) * BS)
        pT = np.ascontiguousarray(person[sl].T)
        hT = np.ascontiguousarray(home[sl].T)
        wT = np.ascontiguousarray(work[sl].T)
        gT = np.ascontiguousarray(purpose[sl].T)
        encCat = np.concatenate([pT, hT, wT, gT], 0)        # [168, BS]
        catC = np.concatenate([pT, wT, hT], 0)              # [160, BS]
        in_maps.append({
            "encCatA": np.ascontiguousarray(encCat[0:128]),
            "encCatB": np.ascontiguousarray(encCat[128:168]),
            "catCA": np.ascontiguousarray(catC[0:128]),
            "catCB": np.ascontiguousarray(catC[128:160]),
            "epsT": np.ascontiguousarray(eps[sl].T),
            **shared,
        })
    return in_maps


_cached = {}


def _get_runner():
    if "runner" not in _cached:
        import jax
        import jax.numpy as jnp
        from jax.sharding import Mesh, PartitionSpec
        from jax.experimental.shard_map import shard_map
        from concourse import bass2jax

        nc = _build_program()
        bass2jax.install_neuronx_cc_hook()
        partition_name = nc.partition_id_tensor.name if nc.partition_id_tensor else None
        in_names, out_names, out_avals = [], [], []
        for alloc in nc.m.functions[0].allocations:
            if not isinstance(alloc, mybir.MemoryLocationSet):
                continue
            name = alloc.memorylocations[0].name
            if alloc.kind == "ExternalInput":
                if name != partition_name:
                    in_names.append(name)
            elif alloc.kind == "ExternalOutput":
                out_names.append(name)
                out_avals.append(jax.core.ShapedArray(
                    tuple(alloc.tensor_shape), mybir.dt.np(alloc.dtype)))
        n_params, n_outs = len(in_names), len(out_names)
        all_in_names = list(in_names) + list(out_names)
        if partition_name is not None:
            all_in_names.append(partition_name)
        devices = jax.devices()[:N_CORES]
        mesh = Mesh(np.asarray(devices), ("core",))

        def _body(*args):
            operands = list(args)
            if partition_name is not None:
                operands.append(bass2jax.partition_id_tensor())
            outs = bass2jax._bass_exec_p.bind(
                *operands,
                out_avals=tuple(out_avals),
                in_names=tuple(all_in_names),
                out_names=tuple(out_names),
                lowering_input_output_aliases=(),
                sim_require_finite=True,
                sim_require_nnan=True,
                nc=nc,
            )
            return tuple(outs)

        fn = jax.jit(shard_map(
            _body, mesh=mesh,
            in_specs=(PartitionSpec("core"),) * (n_params + n_outs),
            out_specs=(PartitionSpec("core"),) * n_outs,
            check_rep=False,
        ))
        sharding = jax.sharding.NamedSharding(mesh, PartitionSpec("core"))
        zmaker = jax.jit(
            lambda: tuple(
                jnp.zeros((N_CORES * av.shape[0], *av.shape[1:]), av.dtype)
                for av in out_avals),
            out_shardings=tuple(sharding for _ in out_avals),
        )
        zeros = zmaker()
        jax.block_until_ready(zeros)
        _cached["runner"] = (fn, zeros, in_names, out_names, out_avals, mesh)
    return _cached["runner"]


def _put_inputs(in_maps):
    import jax
    from jax.sharding import PartitionSpec
    fn, zeros, in_names, out_names, out_avals, mesh = _get_runner()
    sharding = jax.sharding.NamedSharding(mesh, PartitionSpec("core"))
    concat = [
        np.concatenate([np.asarray(m[name]) for m in in_maps], axis=0)
        for name in in_names
    ]
    return [jax.device_put(a, sharding) for a in concat]


def _run_device(dev_in):
    import jax
    fn, zeros, in_names, out_names, out_avals, mesh = _get_runner()
    outs = fn(*dev_in, *zeros)
    jax.block_until_ready(outs)
    return outs


def kernel(**inputs):
    in_maps = _host_prep(inputs)
    last_err = None
    for attempt in range(3):
        try:
            dev_in = _put_inputs(in_maps)
            outs = _run_device(dev_in)
            break
        except Exception as e:  # device can wedge transiently; rebuild + retry
            last_err = e
            _cached.clear()
            import time as _time
            _time.sleep(20 * (attempt + 1))
    else:
        raise last_err
    fn, zeros, in_names, out_names, out_avals, mesh = _get_runner()
    host = {name: np.asarray(o) for name, o in zip(out_names, outs)}
    logits = host["logits"].reshape(N_CORES, BS, T, Z).reshape(B, T, Z)
    muT = host["muT"].reshape(N_CORES, L, BS)
    lvT = host["lvT"].reshape(N_CORES, L, BS)
    mu = np.concatenate([muT[c].T for c in range(N_CORES)], 0)
    lv = np.concatenate([lvT[c].T for c in range(N_CORES)], 0)
    return (logits.astype(np.float32), mu.astype(np.float32), lv.astype(np.float32))
